# revision 4
# baseline (speedup 1.0000x reference)
"""4-D average pool (kernel=2, stride=2) over [2,16,32,32,32,32] f32, on 8 NeuronCores.

Strategy: data-parallel over the 32 (b,c) slices -> 4 slices per core; the
per-core input is a contiguous [4096, 1024] f32 block (rows = (slice,d1,d2),
cols = (d3,d4)).

Partitioning puts ALL four pooled dims in the free dimension, so the whole
reduction runs on DVE adds -- no PE matmul, no PSUM, no ACTIVATE copies:
  partition p = h*64 + slice*16 + o1   (h = d2 bit 4, o1 = d1//2)
  free (per chunk j) = (d1l 2, d2l 2, d3 32, d4 32) with d2 = 16h+2j+d2l
A partition's chunk data is two contiguous 8 KiB row-pairs (one per d1l).
SBUF APs need a plain partition dim, so each 2 MiB chunk loads as TWO 1 MiB
DMAs (h=0 -> partitions 0..63, h=1 -> 64..127); the halves land on disjoint
(even/odd) SDMA engine sets and drain concurrently.  All loads go on the SP
HWDGE ring and the full 16 MiB shard stays SBUF-resident, so no load ever
waits and the stream runs gapless at the per-core HBM rate.

Per chunk DVE pools d4, d3, d2, d1 pairs (4 halving adds) and a
tensor_scalar applies the 1/16.  DVE runs ~50% occupied, so the load stream
stays the critical path.  The last chunk is further split by d1l into two
1 MiB sub-blocks so only a ~1.5 us DVE chain + one 128 KiB store trail the
final load, and the ~75-instruction kernel keeps the iram-load preamble and
event-semaphore teardown short.
"""

import sys

import numpy as np

if "/opt/trn_rl_repo" not in sys.path:
    sys.path.insert(0, "/opt/trn_rl_repo")

import concourse.bacc as bacc
import concourse.bass as bass
import concourse.tile as tile
from concourse import mybir
from concourse.bass_utils import run_bass_kernel_spmd

N_CORES = 8
SLICES_PER_CORE = 4  # 32 (b,c) slices / 8 cores
ROWS = SLICES_PER_CORE * 1024  # 4096
F32 = mybir.dt.float32


def build_nc() -> bass.Bass:
    # Bacc (not raw Bass): its compile() splits multi-sem sync waits into
    # event-semaphore instructions (TRN2 allows one wait per instruction).
    nc = bacc.Bacc()
    x = nc.dram_tensor("x", [ROWS, 1024], F32, kind="ExternalInput")
    y = nc.dram_tensor("y", [ROWS // 4, 256], F32, kind="ExternalOutput")

    # DRAM row = s*1024 + (2*o1+d1l)*32 + 16*h + 2*j + d2l; partition (h,s,o1)
    xv = x.rearrange(
        "(s o1 d1l h jr d2l) c -> h (s o1) d1l jr (d2l c)",
        s=4, o1=16, d1l=2, h=2, jr=8, d2l=2,
    )
    # y row = s*256 + o1*16 + 8*h + j
    yv = y.rearrange(
        "(s o1 hh o2r) c -> hh (s o1) o2r c", s=4, o1=16, hh=2, o2r=8
    )

    with tile.TileContext(nc) as tc:
        with (
            # bufs = chunk count -> every chunk gets its own slot; the whole
            # 16 MiB shard is SBUF-resident so load DMAs never wait
            tc.tile_pool(name="inp", bufs=7) as inp,
            tc.tile_pool(name="inh", bufs=2) as inh,
            tc.tile_pool(name="m1p", bufs=2) as m1p,
            tc.tile_pool(name="m2p", bufs=2) as m2p,
            tc.tile_pool(name="m3p", bufs=4) as m3p,
            tc.tile_pool(name="m4p", bufs=2) as m4p,
            tc.tile_pool(name="obp", bufs=4) as obp,
        ):

            def pool_d4_d3(tv, nd1l, mtag):
                # tv free = (d1l nd1l, d2l 2, d3 32, d4 32)
                a = nd1l * 2 * 32
                v = tv.rearrange("p (a o4 e4) -> p a o4 e4", a=a, o4=16)
                m1 = m1p.tile([128, a * 16], F32, tag=f"{mtag}1")
                m1v = m1[:].rearrange("p (a o4) -> p a o4", a=a)
                nc.vector.tensor_add(m1v, v[:, :, :, 0], v[:, :, :, 1])
                b = nd1l * 2
                w = m1[:].rearrange(
                    "p (b o3 e3 o4) -> p b o3 e3 o4", b=b, o3=16, e3=2
                )
                m2 = m2p.tile([128, b * 256], F32, tag=f"{mtag}2")
                m2v = m2[:].rearrange("p (b o3 o4) -> p b o3 o4", b=b, o3=16)
                nc.vector.tensor_add(m2v, w[:, :, :, 0, :], w[:, :, :, 1, :])
                return m2

            def finish_and_store(m4, j):
                ob = obp.tile([128, 256], F32, tag="ob")
                nc.vector.tensor_scalar_mul(ob[:], m4[:], 1.0 / 16.0)
                # ACT ring so store triggers never block the load stream;
                # one DMA per h half (SBUF APs need a plain partition dim)
                for h in range(2):
                    nc.scalar.dma_start(
                        yv[h, :, j, :], ob[:][64 * h : 64 * h + 64, :]
                    )

            for j in range(7):  # full 2 MiB chunks
                t = inp.tile([128, 4096], F32, tag="t")
                for h in range(2):
                    nc.sync.dma_start(
                        t[:][64 * h : 64 * h + 64, :].rearrange(
                            "q (d1l c) -> q d1l c", d1l=2
                        ),
                        xv[h, :, :, j, :],
                    )
                m2 = pool_d4_d3(t[:], 2, "f")
                # pool d2l pairs: free (d1l 2, e2 2, c 256)
                z = m2[:].rearrange("p (d1l e2 c) -> p d1l e2 c", d1l=2, e2=2)
                m3 = m3p.tile([128, 512], F32, tag="m3f")
                m3v = m3[:].rearrange("p (d1l c) -> p d1l c", d1l=2)
                nc.vector.tensor_add(m3v, z[:, :, 0, :], z[:, :, 1, :])
                # pool the d1l pair
                zz = m3[:].rearrange("p (d1l c) -> p d1l c", d1l=2)
                m4 = m4p.tile([128, 256], F32, tag="m4")
                nc.vector.tensor_add(m4[:], zz[:, 0, :], zz[:, 1, :])
                finish_and_store(m4, j)

            # last chunk split by d1l: only a short DVE chain + one store
            # trail the final 512 KiB load
            m3s = []
            for g in range(2):
                t = inh.tile([128, 2048], F32, tag="th")
                for h in range(2):
                    nc.sync.dma_start(
                        t[:][64 * h : 64 * h + 64, :], xv[h, :, g, 7, :]
                    )
                m2 = pool_d4_d3(t[:], 1, "h")
                z = m2[:].rearrange("p (e2 c) -> p e2 c", e2=2)
                m3 = m3p.tile([128, 256], F32, tag="m3h")
                nc.vector.tensor_add(m3[:], z[:, 0, :], z[:, 1, :])
                m3s.append(m3)
            m4 = m4p.tile([128, 256], F32, tag="m4")
            nc.vector.tensor_add(m4[:], m3s[0][:], m3s[1][:])
            finish_and_store(m4, 7)

    nc.compile()
    return nc


_NC_CACHE: bass.Bass | None = None


def kernel(nd_tensor: np.ndarray, _trace: bool = False):
    global _NC_CACHE
    x = np.ascontiguousarray(np.asarray(nd_tensor, dtype=np.float32)).reshape(
        32, 1024, 1024
    )
    if _NC_CACHE is None:
        _NC_CACHE = build_nc()
    nc = _NC_CACHE

    in_maps = [
        {
            "x": np.ascontiguousarray(
                x[SLICES_PER_CORE * i : SLICES_PER_CORE * (i + 1)]
            ).reshape(ROWS, 1024),
        }
        for i in range(N_CORES)
    ]
    res = run_bass_kernel_spmd(
        nc, in_maps, core_ids=list(range(N_CORES)), trace=_trace
    )
    out = np.stack([res.results[i]["y"] for i in range(N_CORES)])  # [8,1024,256]
    out = out.reshape(2, 16, 16, 16, 16, 16).astype(np.float32)
    if _trace:
        kernel.last_results = res
    return out
